# revision 25
# baseline (speedup 1.0000x reference)
"""GRU encoder kernel for Trainium2 (8 NeuronCores, data-parallel over batch).

Problem: nn_Encoder (B=64, T=2048, E=256, H=512, V=32000)
  lengths = count(X != 0, per row)
  Xemb = emb[X]
  xr/xz/xh = Xemb @ W{r,z,h}.T + b      (input-side projections)
  GRU recurrence over t with update mask (t < length)
  out = tanh(h_T @ V_w.T + V_b)

Design (per core, local batch BL=8):
  - Input staging is minimized (the dominant cost of the naive kernel):
    the 32MB f32 emb table is NOT replicated 8x. Each core receives a 1/8
    shard in bf16 (2.05MB) plus a 1/8 shard of a bf16 weight blob
    [uT;wT;vT] (360KB); both are AllGathered on-device over NeuronLink
    into Shared DRAM. Total staged bytes: 2.64MB/core vs 37.3MB/core.
  - Masking trick (z-gate NEGATED end to end): the kernel's sigmoid
    computes z' = 1-z, and xz~ gets -1e9 for t>=length_b so z'==0 exactly
    -> h'=h, no per-step masking. Negation folds into Wz/Uz/bias on host.
  - Phase 1: gather (indirect DMA, bf16) -> PE-transpose Xemb to [E, tok]
    -> project with stationary bf16 W.T (FWL fast weight loads) into the
    TRANSPOSED layout [H, tok]; biases and the -1e9 mask are folded in as
    extra K=1 rank-1 matmuls accumulated into the same PSUM. Results stored
    bf16 in a DRAM scratch laid out so the recurrence reads are strided APs.
  - Phase 2 (recurrence): transposed layout throughout. h kept as
    hT [128, (k,b)] (fp32 master + bf16 matmul copy, both written directly
    by the blend adds). Per step: 48 weight-stationary bf16 matmuls
    [128,128]x[128,8] + 2 identity-matmuls injecting xr/xz~/xh (both issued
    before the sigmoid wait), sigmoid/tanh on ACT, and the blend arranged
    for minimum critical path: m1=z'h and zh=h-z'h run on DVE while PE does
    the upd matmuls; after tanh only mul+add remain before hbf is ready.
    Time loop = hardware For_i over chunks, 256-step unrolled body.
  - Head: out.T = tanh(V_w @ h + V_b) via 16 bf16 matmuls,
    DMA [128, 32] per core; host reassembles the 8 cores.
Measured (A/B differential wall-clock on 8 axon trn2 cores): recurrence
~7.4ms, phase1+AllGather+head ~1.2ms; staged input bytes 21.1MB total.
"""

import numpy as np
import ml_dtypes

B, T, E, H, V = 64, 2048, 256, 512, 32000
NCORES = 8
BL = B // NCORES          # 8 batch rows per core
CT = 256                  # timesteps per chunk
P = 128

_BUILD_CACHE = {}


def _build(nch, debug=False, kk=4, mode="full", unroll=False, rec_mult=1):
    """Build + compile the per-core Bass program for nch chunks of CT steps.

    kk/mode are timing-ablation knobs: kk = K-tiles per gate matmul (4 = full);
    mode in {"full", "nop1" (skip phase 1), "norec" (skip recurrence)}.
    unroll=True replaces the For_i hardware loop with a Python loop (for
    TimelineSim, which cannot resolve register-mode branches).
    """
    import concourse.bass as bass
    import concourse.mybir as mybir
    import concourse.tile as tile
    from concourse import bacc
    from concourse.bass import ds
    from concourse.tile_rust import add_dep_helper

    dt = mybir.dt
    AF = mybir.ActivationFunctionType
    OP = mybir.AluOpType
    IOA = bass.IndirectOffsetOnAxis

    t_total = nch * CT
    ngrp = (BL * t_total) // P        # gather groups of 128 tokens
    gpc = (BL * CT) // P              # gather groups per chunk (16)
    CHE = 3 * 4 * 4 * 2 * CT          # elements per chunk in xscr (=24576)
    NS = (BL * CT) // 512             # 512-token slices per chunk (4)

    nc = bacc.Bacc("TRN2", target_bir_lowering=False, debug=False,
                   num_devices=NCORES)

    # ---- DRAM I/O ----
    if mode != "nop1":
        # emb arrives sharded 1/8 per core (bf16) and is AllGathered on-device
        embsh_d = nc.dram_tensor("embsh", [V // NCORES, E], dt.bfloat16,
                                 kind="ExternalInput")
        embb_d = nc.dram_tensor("embb", [V // NCORES, E], dt.bfloat16)
        embf_d = nc.dram_tensor("embf", [V, E], dt.bfloat16,
                                addr_space="Shared")
        xidx_d = nc.dram_tensor("xidx", [P, ngrp], dt.int32, kind="ExternalInput")
        xbt_d = nc.dram_tensor("xbt", [BL, t_total], dt.int32, kind="ExternalInput")
        iota_d = nc.dram_tensor("iota", [BL, t_total], dt.float32, kind="ExternalInput")
        bias_d = nc.dram_tensor("bias", [1, 3 * H], dt.bfloat16, kind="ExternalInput")
    # shared weight blob [uT(3H); wT(3E); vT(H)] sharded 1/8 per core + AllGather
    WBR = 3 * H + 3 * E + H  # 2816 rows
    wbsh_d = nc.dram_tensor("wbsh", [WBR // NCORES, H], dt.bfloat16,
                            kind="ExternalInput")
    wbb_d = nc.dram_tensor("wbb", [WBR // NCORES, H], dt.bfloat16)
    wbf_d = nc.dram_tensor("wbf", [WBR, H], dt.bfloat16, addr_space="Shared")
    uT_d = wbf_d
    wT_d = wbf_d  # offset by 3*H rows at use sites
    vT_d = wbf_d  # offset by 3*H + 3*E rows
    WOF = 3 * H
    VOF = 3 * H + 3 * E
    vb_d = nc.dram_tensor("vb", [P, 4], dt.float32, kind="ExternalInput")
    eyeb_d = nc.dram_tensor("eyeb", [P, P], dt.bfloat16, kind="ExternalInput")
    out_d = nc.dram_tensor("out", [P, 32], dt.float32, kind="ExternalOutput")
    xscr_d = nc.dram_tensor("xscr", [P, nch * CHE], dt.bfloat16,
                            kind="ExternalOutput" if debug else "Internal")
    if debug:
        hlog_d = nc.dram_tensor("hlog", [P, nch * 32], dt.float32,
                                kind="ExternalOutput")

    store_insts = []

    with tile.TileContext(nc) as tc:
        with (
            tc.tile_pool(name="const", bufs=1) as cp,
            tc.tile_pool(name="state", bufs=1) as sp,
        ):
            # ---- weight blob AllGather, then persistent consts ----
            wbi = nc.sync.dma_start(wbb_d[:], wbsh_d[:])
            wag = nc.gpsimd.collective_compute(
                "AllGather", mybir.AluOpType.bypass,
                replica_groups=[list(range(NCORES))],
                ins=[wbb_d.ap().opt()], outs=[wbf_d.ap().opt()])
            add_dep_helper(wag.ins, wbi.ins, sync=True, reason="wb bounce->AG")

            def _wb_load(tile_ap, row0, row1):
                li = nc.sync.dma_start(tile_ap, wbf_d[row0:row1, :])
                add_dep_helper(li.ins, wag.ins, sync=True, reason="AG->load")

            uT_sb = {}
            for g in range(3):
                for k in range(4):
                    tl_ = cp.tile([P, H], dt.bfloat16, tag=f"uT{g}{k}")
                    _wb_load(tl_[:], g * H + k * P, g * H + (k + 1) * P)
                    uT_sb[(g, k)] = tl_
            vT_sb = {}
            for k in range(4):
                tl_ = cp.tile([P, H], dt.bfloat16, tag=f"vT{k}")
                _wb_load(tl_[:], VOF + k * P, VOF + (k + 1) * P)
                vT_sb[k] = tl_
            vb_sb = cp.tile([P, 4], dt.float32, tag="vb")
            nc.sync.dma_start(vb_sb[:], vb_d[:])
            eyeb = cp.tile([P, P], dt.bfloat16, tag="eyeb")
            nc.sync.dma_start(eyeb[:], eyeb_d[:])

            # ---- recurrence state ----
            h32 = sp.tile([P, 32], dt.float32, tag="h32")
            hbf = sp.tile([P, 32], dt.bfloat16, tag="hbf")
            nc.vector.memset(h32[:], 0.0)
            nc.vector.memset(hbf[:], 0.0)
            xc = sp.tile([P, CHE], dt.bfloat16, tag="xc")

            # ================= PHASE 1: gather + project =================
            def _phase1():
              with (
                tc.tile_pool(name="p1c", bufs=1) as pc1,
                tc.tile_pool(name="p1g", bufs=1) as p1,
                tc.tile_pool(name="p1x", bufs=4) as px,
                tc.tile_pool(name="p1ps", bufs=2, space="PSUM") as pst,
                tc.tile_pool(name="p2ps", bufs=4, space="PSUM") as psp,
              ):
                # emb shard -> bounce -> AllGather to full table (bf16)
                bi = nc.sync.dma_start(embb_d[:], embsh_d[:])
                ag = nc.gpsimd.collective_compute(
                    "AllGather", mybir.AluOpType.bypass,
                    replica_groups=[list(range(NCORES))],
                    ins=[embb_d.ap().opt()], outs=[embf_d.ap().opt()])
                add_dep_helper(ag.ins, bi.ins, sync=True, reason="emb bounce->AG")
                # phase-1-only consts
                wT_sb = {}
                for g in range(3):
                    for k in range(2):
                        tl_ = pc1.tile([P, H], dt.bfloat16, tag=f"wT{g}{k}")
                        _wb_load(tl_[:], WOF + g * E + k * P,
                                 WOF + g * E + (k + 1) * P)
                        wT_sb[(g, k)] = tl_
                ones_sb = pc1.tile([1, H], dt.bfloat16, tag="ones")
                nc.vector.memset(ones_sb[:], 1.0)
                onesb = ones_sb
                bias_sb = pc1.tile([1, 3 * H], dt.bfloat16, tag="bias")
                nc.sync.dma_start(bias_sb[:], bias_d[:])
                xidx_sb = pc1.tile([P, ngrp], dt.int32, tag="xidx")
                nc.sync.dma_start(xidx_sb[:], xidx_d[:])
                xbt_sb = pc1.tile([BL, t_total], dt.int32, tag="xbt")
                nc.sync.dma_start(xbt_sb[:], xbt_d[:])
                iota_sb = pc1.tile([BL, t_total], dt.float32, tag="iota")
                nc.sync.dma_start(iota_sb[:], iota_d[:])

                # lengths + mask (per-partition ops in [b, t] layout)
                nz_sb = pc1.tile([BL, t_total], dt.float32, tag="nz")
                nc.vector.tensor_scalar(out=nz_sb[:], in0=xbt_sb[:], scalar1=0,
                                        scalar2=None, op0=OP.not_equal)
                len_sb = pc1.tile([BL, 1], dt.float32, tag="len")
                nc.vector.tensor_reduce(out=len_sb[:], in_=nz_sb[:], op=OP.add,
                                        axis=mybir.AxisListType.X)
                maskbig = pc1.tile([BL, t_total], dt.bfloat16, tag="maskbig")
                nc.vector.tensor_scalar(out=maskbig[:], in0=iota_sb[:],
                                        scalar1=len_sb[:, 0:1], scalar2=-1.0e9,
                                        op0=OP.is_ge, op1=OP.mult)

                for ch in range(nch):
                    xemb = p1.tile([P, gpc * E], dt.bfloat16, tag="xemb")
                    for gl in range(gpc):
                        gi = nc.gpsimd.indirect_dma_start(
                            out=xemb[:, gl * E:(gl + 1) * E],
                            out_offset=None,
                            in_=embf_d[:],
                            in_offset=IOA(ap=xidx_sb[:, ch * gpc + gl: ch * gpc + gl + 1], axis=0),
                        )
                        add_dep_helper(gi.ins, ag.ins, sync=True,
                                       reason="AG before gather")
                    # per-chunk mask row on partition 0 (slices for matmul rhs)
                    mask1c = px.tile([1, BL * CT], dt.bfloat16, tag="mask1c")
                    for b in range(BL):
                        nc.sync.dma_start(mask1c[0:1, b * CT:(b + 1) * CT],
                                          maskbig[b:b + 1, ch * CT:(ch + 1) * CT])
                    # transpose Xemb -> [E, tok] (two e-halves)
                    xembT = p1.tile([P, 2 * BL * CT], dt.bfloat16, tag="xembT")
                    for gl in range(gpc):
                        for eh in range(2):
                            tp = pst.tile([P, P], dt.bfloat16, tag="tp")
                            nc.tensor.transpose(
                                tp[:], xemb[:, gl * E + eh * P: gl * E + eh * P + P],
                                eyeb[:])
                            nc.any.tensor_copy(
                                xembT[:, eh * BL * CT + gl * P: eh * BL * CT + (gl + 1) * P],
                                tp[:])
                    # projections: out [H-part, tok] per (gate, m, ns)
                    for g in range(3):
                        for m in range(4):
                            for ns in range(NS):
                                pp = psp.tile([P, 512], dt.float32, tag="pp")
                                for k in range(2):
                                    nc.tensor.matmul(
                                        pp[:],
                                        lhsT=wT_sb[(g, k)][:, m * P:(m + 1) * P],
                                        rhs=xembT[:, k * BL * CT + ns * 512:
                                                  k * BL * CT + ns * 512 + 512],
                                        start=(k == 0), stop=False)
                                # bias row (K=1): out[j, n] += bias[j] * 1
                                nc.tensor.matmul(
                                    pp[:],
                                    lhsT=bias_sb[0:1, g * H + m * P: g * H + (m + 1) * P],
                                    rhs=ones_sb[0:1, 0:512],
                                    start=False, stop=(g != 1))
                                if g == 1:  # z-gate: += 1e9 mask, two b-halves
                                    for b2 in range(2):
                                        b = 2 * ns + b2
                                        nc.tensor.matmul(
                                            pp[:, b2 * CT:(b2 + 1) * CT],
                                            lhsT=onesb[0:1, 0:P],
                                            rhs=mask1c[0:1, b * CT: b * CT + CT],
                                            start=False, stop=True)
                                xp = px.tile([P, 512], dt.bfloat16, tag="xp")
                                nc.any.tensor_copy(xp[:], pp[:])
                                off = ch * CHE + (g * 4 + m) * (4 * 512) + ns * 512
                                si = nc.sync.dma_start(xscr_d[:, off:off + 512], xp[:])
                                store_insts.append(si)

            if mode != "nop1":
                _phase1()

            # ================= PHASE 2: recurrence =================
            with (
                tc.tile_pool(name="r_sb", bufs=2) as rp,
                tc.tile_pool(name="rAps", bufs=2, space="PSUM") as psA,
                tc.tile_pool(name="rBps", bufs=2, space="PSUM") as psB,
            ):
                def _rec_body(ci):
                    li = nc.sync.dma_start(xc[:], xscr_d[:, ds(ci * CHE, CHE)])
                    for s in store_insts:
                        add_dep_helper(li.ins, s.ins, sync=True, reason="xscr RAW")
                    # free layout of xc: [g(3), mns(16), b2(2), tl(CT)]
                    xc5 = xc[:].rearrange("p (g mns b2 tl) -> p g mns b2 tl",
                                          g=3, mns=16, b2=2, tl=CT)
                    for tl_ in range(CT):
                        # z-gate is NEGATED end-to-end (weights, bias, mask),
                        # so sigmoid gives z' = 1-z and h' = (h - z'h) + z'u.
                        pA = psA.tile([P, 64], dt.float32, tag="pA")
                        nc.tensor.matmul(pA[:], lhsT=eyeb[:],
                                         rhs=xc5[:, 0:2, :, :, tl_:tl_ + 1],
                                         start=True, stop=(kk == 0))
                        for g in range(2):
                            for m in range(4):
                                for k in range(kk):
                                    nc.tensor.matmul(
                                        pA[:, g * 32 + m * 8: g * 32 + (m + 1) * 8],
                                        lhsT=uT_sb[(g, k)][:, m * P:(m + 1) * P],
                                        rhs=hbf[:, 8 * k: 8 * k + 8],
                                        start=False, stop=(k == kk - 1))
                        pB = psB.tile([P, 32], dt.float32, tag="pB")
                        nc.tensor.matmul(pB[:], lhsT=eyeb[:],
                                         rhs=xc5[:, 2:3, :, :, tl_:tl_ + 1],
                                         start=True, stop=(kk == 0))
                        rz = rp.tile([P, 64], dt.float32, tag="rz")
                        nc.scalar.activation(rz[:], pA[:], AF.Sigmoid)
                        rh = rp.tile([P, 32], dt.bfloat16, tag="rh")
                        nc.vector.tensor_mul(rh[:], rz[:, 0:32], h32[:])
                        # off-chain while PE runs the upd matmuls:
                        m1 = rp.tile([P, 32], dt.float32, tag="m1")
                        nc.vector.tensor_mul(m1[:], rz[:, 32:64], h32[:])
                        zh = rp.tile([P, 32], dt.float32, tag="zh")
                        nc.vector.tensor_sub(zh[:], h32[:], m1[:])
                        for m in range(4):
                            for k in range(kk):
                                nc.tensor.matmul(
                                    pB[:, m * 8:(m + 1) * 8],
                                    lhsT=uT_sb[(2, k)][:, m * P:(m + 1) * P],
                                    rhs=rh[:, 8 * k: 8 * k + 8],
                                    start=False, stop=(k == kk - 1))
                        uu = rp.tile([P, 32], dt.float32, tag="uu")
                        nc.scalar.activation(uu[:], pB[:], AF.Tanh)
                        ee = rp.tile([P, 32], dt.float32, tag="ee")
                        nc.vector.tensor_mul(ee[:], rz[:, 32:64], uu[:])
                        nc.vector.tensor_add(hbf[:], zh[:], ee[:])
                        nc.vector.tensor_add(h32[:], zh[:], ee[:])
                    if debug:
                        nc.sync.dma_start(hlog_d[:, ds(ci * 32, 32)], h32[:])

                if mode != "norec":
                    if unroll:
                        for ci in range(nch):
                            _rec_body(ci)
                    else:
                        for _rep in range(rec_mult):
                            with tc.For_i(0, nch) as ci:
                                _rec_body(ci)

                # ---- head: out.T = tanh(V_w @ h + V_b) ----
                pO = psA.tile([P, 32], dt.float32, tag="pO")
                for m in range(4):
                    for k in range(4):
                        nc.tensor.matmul(
                            pO[:, m * 8:(m + 1) * 8],
                            lhsT=vT_sb[k][:, m * P:(m + 1) * P],
                            rhs=hbf[:, 8 * k: 8 * k + 8],
                            start=(k == 0), stop=(k == 3))
                ob = rp.tile([P, 32], dt.float32, tag="ob")
                for m in range(4):
                    nc.scalar.activation(ob[:, m * 8:(m + 1) * 8],
                                         pO[:, m * 8:(m + 1) * 8],
                                         AF.Tanh, bias=vb_sb[:, m:m + 1])
                nc.sync.dma_start(out_d[:], ob[:])

    nc.compile()
    return nc


def _prep_inputs(X, emb, Wr_w, Wr_b, Ur_w, Ur_b, Wz_w, Wz_b, Uz_w, Uz_b,
                 Wxh_w, Wxh_b, Whh_w, Whh_b, V_w, V_b, nch):
    t_total = nch * CT
    bf16 = ml_dtypes.bfloat16
    f32 = np.float32

    # z-gate fully negated so the kernel's sigmoid yields z' = 1-z
    bias = np.concatenate([Wr_b + Ur_b, -(Wz_b + Uz_b), Wxh_b + Whh_b]) \
        .reshape(1, 3 * H).astype(bf16)
    # weight blob [uT(3H); wT(3E); vT(H)] rows, [*, H] bf16
    wblob = np.concatenate(
        [np.ascontiguousarray(u.T) for u in (Ur_w, -Uz_w, Whh_w)]
        + [np.ascontiguousarray(w.T) for w in (Wr_w, -Wz_w, Wxh_w)]
        + [np.ascontiguousarray(V_w.T)], axis=0).astype(bf16)  # [2816, H]
    WBR = wblob.shape[0]
    vb = np.ascontiguousarray(V_b.reshape(4, P).T).astype(f32)  # vb[p,m]
    eyeb = np.eye(P, dtype=f32).astype(bf16)
    iota = np.broadcast_to(np.arange(t_total, dtype=f32), (BL, t_total)).copy()
    embh = np.ascontiguousarray(emb).astype(bf16)
    EVC = V // NCORES

    in_maps = []
    for c in range(NCORES):
        Xc = np.asarray(X[c * BL:(c + 1) * BL, :t_total])
        # token order n' = ch*(BL*CT) + b*CT + tl
        arr = np.ascontiguousarray(
            Xc.reshape(BL, nch, CT).transpose(1, 0, 2).reshape(-1))
        xidx = np.ascontiguousarray(
            arr.reshape(-1, P).T).astype(np.int32)             # [p, g]
        xbt = np.ascontiguousarray(Xc).astype(np.int32)
        wsh = WBR // NCORES
        in_maps.append(dict(
            embsh=np.ascontiguousarray(embh[c * EVC:(c + 1) * EVC]),
            wbsh=np.ascontiguousarray(wblob[c * wsh:(c + 1) * wsh]),
            xidx=xidx, xbt=xbt, iota=iota, bias=bias,
            vb=vb, eyeb=eyeb))
    return in_maps


def _run(in_maps, nch, trace=False):
    from concourse.bass_utils import run_bass_kernel_spmd
    if nch not in _BUILD_CACHE:
        _BUILD_CACHE[nch] = _build(nch)
    nc = _BUILD_CACHE[nch]
    res = run_bass_kernel_spmd(nc, in_maps, core_ids=list(range(NCORES)),
                               trace=trace)
    # per-core out is outT [128 p, 32 (k,b)] with out[b, 128k+p] = outT[p, 8k+b]
    outs = []
    for c in range(NCORES):
        ot = np.asarray(res.results[c]["out"])             # [128, 32]
        o = ot.reshape(P, 4, BL).transpose(2, 1, 0).reshape(BL, H)
        outs.append(o)
    return np.concatenate(outs, axis=0).astype(np.float32), res


def kernel(X, emb, Wr_w, Wr_b, Ur_w, Ur_b, Wz_w, Wz_b, Uz_w, Uz_b,
           Wxh_w, Wxh_b, Whh_w, Whh_b, V_w, V_b):
    nch = T // CT
    in_maps = _prep_inputs(
        X, emb, Wr_w, Wr_b, Ur_w, Ur_b, Wz_w, Wz_b, Uz_w, Uz_b,
        Wxh_w, Wxh_b, Whh_w, Whh_b, V_w, V_b, nch)
    out, _ = _run(in_maps, nch)
    return out

